# revision 18
# baseline (speedup 1.0000x reference)
"""Self-contained Trainium2 Bass kernel for nn_CrossStageAttention.

Data-parallel over batch: 16 images -> 8 NeuronCores x 2 images each.
Training-mode BatchNorm statistics are made global via two tiny AllReduces.

All heavy matmuls run as float32r on the PE array. The torch
"(attn@v).transpose(1,2).reshape" scramble is absorbed into the fuse access
patterns (o_nat orientation): catT[i, pos=2u+v] = o_nat[512v+i, u].

v2: the whole post-attention pipeline is channel-on-partition:
 - fusx / xT stay in SBUF (no DRAM spills)
 - conv input is one 2D-padded buffer x2T[c, (h+1)*34 + (w+1)]
 - conv output yT[c_out, pos] via lhsT=ow, rhs=x2T window; BN2 stats fold
   into the PSUM->SBUF copies (Act accum_out / DVE scalar_tensor_tensor)
 - BN2 apply is one per-partition Act relu(scale*y+bias) per channel chunk
 - final PE transposes produce the natural [pos, C] output
"""
import numpy as np
from contextlib import ExitStack

import concourse.bass as bass
import concourse.tile as tile
import concourse.bacc as bacc
from concourse import mybir, masks
from concourse.bass_utils import run_bass_kernel_spmd

N_CORES = 8
IMGS = 2
C = 512
N = 1024          # query positions per image (32x32)
PC = 256
MP = 4096         # prev positions per image (64x64)
F32 = mybir.dt.float32
F32R = mybir.dt.float32r
BF16 = mybir.dt.bfloat16
SCALE = 32 ** -0.5
B0_SELF = 128.0   # constant softmax-stabilization bias for self-attention
EPS = 1e-5
INV_CNT = 1.0 / (16 * 1024)
AF = mybir.ActivationFunctionType
ALU = mybir.AluOpType
X_AXIS = mybir.AxisListType.X


def build_nc():
    nc = bacc.Bacc("TRN2", target_bir_lowering=False, debug=False,
                   num_devices=N_CORES)
    x_d = nc.dram_tensor("x", [IMGS, N, C], F32R, kind="ExternalInput").ap()
    px_d = nc.dram_tensor("px", [IMGS, MP, PC], F32R, kind="ExternalInput").ap()
    wq_d = nc.dram_tensor("wq", [C, C], F32R, kind="ExternalInput").ap()
    wp_d = nc.dram_tensor("wp", [PC, C], F32R, kind="ExternalInput").ap()
    fw_d = nc.dram_tensor("fw", [2 * C, C], F32R, kind="ExternalInput").ap()
    ow_d = nc.dram_tensor("ow", [9, C, C], F32R, kind="ExternalInput").ap()
    g1_d = nc.dram_tensor("g1", [128, 4], F32, kind="ExternalInput").ap()
    b1_d = nc.dram_tensor("b1", [128, 4], F32, kind="ExternalInput").ap()
    g2_d = nc.dram_tensor("g2", [128, 4], F32, kind="ExternalInput").ap()
    b2_d = nc.dram_tensor("b2", [128, 4], F32, kind="ExternalInput").ap()
    pars_d = nc.dram_tensor("pars", [1, 2], F32, kind="ExternalInput").ap()
    out_d = nc.dram_tensor("out", [IMGS, N, C], F32, kind="ExternalOutput").ap()

    with tile.TileContext(nc) as tc, ExitStack() as ctx:
        const = ctx.enter_context(tc.tile_pool(name="const", bufs=1))
        per = ctx.enter_context(tc.tile_pool(name="per", bufs=1))
        scr = ctx.enter_context(tc.tile_pool(name="scr", bufs=10))   # [128,512] scratch
        ld = ctx.enter_context(tc.tile_pool(name="ld", bufs=3))
        sm = ctx.enter_context(tc.tile_pool(name="sm", bufs=10))
        ps = ctx.enter_context(tc.tile_pool(name="ps", bufs=8, space="PSUM"))
        dram = ctx.enter_context(tc.tile_pool(name="dram", bufs=1, space="DRAM"))

        # ------------- DRAM scratch (collective payloads only) -------------
        bn1_in = dram.tile([128, 8], F32, tag="bn1i")
        bn1_all = dram.tile([128 * N_CORES, 8], F32, tag="bn1o")
        bn2_in = dram.tile([128, 8], F32, tag="bn2i")
        bn2_all = dram.tile([128 * N_CORES, 8], F32, tag="bn2o")

        # ------------- persistent cross-scope tensors -------------
        xT_d = dram.tile([IMGS, 4, 128, N], F32R, tag="xT_d")
        fusx_s = [per.tile([128, 4, 2, 512], BF16, tag=f"fusx{i}",
                           name=f"fusx{i}") for i in range(IMGS)]  # 2 MB

        # ------------- constants / params -------------
        identF = const.tile([128, 128], F32, tag="identF")
        masks.make_identity(nc, identF[:])
        ident = const.tile([128, 128], F32R, tag="ident")
        nc.vector.tensor_copy(ident[:], identF[:])
        onesF = const.tile([128, 2], F32, tag="onesF")
        nc.gpsimd.memset(onesF[:], 1.0)
        ones2 = const.tile([128, 2], F32R, tag="ones2")
        nc.vector.tensor_copy(ones2[:], onesF[:])
        b0s = const.tile([128, 1], F32, tag="b0s")
        nc.gpsimd.memset(b0s[:], -B0_SELF)
        eps_t = const.tile([128, 1], F32, tag="eps")
        nc.gpsimd.memset(eps_t[:], EPS)
        g1_s = const.tile([128, 4], F32, tag="g1")
        b1_s = const.tile([128, 4], F32, tag="b1")
        g2_s = const.tile([128, 4], F32, tag="g2")
        b2_s = const.tile([128, 4], F32, tag="b2")
        pars_s = const.tile([1, 2], F32, tag="pars")
        pars_bc = const.tile([128, 2], F32, tag="parsbc")
        s1acc = const.tile([128, 4, 4], F32, tag="s1acc")
        ss1acc = const.tile([128, 4, 4], F32, tag="ss1acc")
        s2acc = const.tile([128, 4, 4], F32, tag="s2acc")
        ss2acc = const.tile([128, 4, 4], F32, tag="ss2acc")
        s1v = const.tile([128, 4], F32, tag="s1v")
        t1v = const.tile([128, 4], F32, tag="t1v")
        s2v = const.tile([128, 4], F32, tag="s2v")
        t2v = const.tile([128, 4], F32, tag="t2v")
        nc.sync.dma_start(g1_s[:], g1_d)
        nc.sync.dma_start(b1_s[:], b1_d)
        nc.sync.dma_start(g2_s[:], g2_d)
        nc.sync.dma_start(b2_s[:], b2_d)
        nc.sync.dma_start(pars_s[:], pars_d)
        nc.gpsimd.partition_broadcast(pars_bc[:], pars_s[:])

        def transpose_to(dst_ap, src_ap, eng):
            pt = ps.tile([128, 512], F32R, tag="b", name="tp")
            nc.tensor.transpose(pt[:, 0:128], src_ap, ident[:])
            if eng == "act":
                nc.scalar.copy(dst_ap, pt[:, 0:128])
            else:
                nc.vector.tensor_copy(dst_ap, pt[:, 0:128])

        # =================== attention scope ===================
        with tc.tile_pool(name="attn", bufs=1) as ap_:
            wq_s = ap_.tile([128, 4, C], F32R, tag="wq", name="wq")
            wp_s = ap_.tile([128, 2, C], F32R, tag="wp", name="wp")
            fw_s = ap_.tile([128, 8, C], F32R, tag="fw", name="fw")
            nc.sync.dma_start(wq_s[:], wq_d.rearrange("(ic p) c -> p ic c", p=128))
            nc.sync.dma_start(wp_s[:], wp_d.rearrange("(ic p) c -> p ic c", p=128))
            nc.sync.dma_start(fw_s[:], fw_d.rearrange("(ic p) o -> p ic o", p=128))
            qT_t = None
            for img in range(IMGS):
                qT_t = ap_.tile([128, 4, N], F32R, tag="qT", name="qT")
                xnow_t = ap_.tile([128, 8, C], F32R, tag="xnow", name="xnow")
                xprev_t = ap_.tile([128, 8, C], F32R, tag="xprev", name="xprev")

                def do_attn(kind, kvT, vnat, nhs=(0, 1)):
                    bias = b0s[:] if kind == "self" else 0.0
                    scl = SCALE * 0.25 if kind == "avg" else SCALE
                    for nh in nhs:
                        eas = []
                        for mi in range(8):
                            lg = ps.tile([128, 512], F32, tag="b", name="lg")
                            for ci in range(4):
                                nc.tensor.matmul(
                                    lg[:],
                                    kvT[:, ci, 128 * mi:128 * mi + 128],
                                    qT_t[:, ci, 512 * nh:512 * nh + 512],
                                    start=(ci == 0), stop=(ci == 3))
                            ea = scr.tile([128, 512], F32R, tag="s", name="ea")
                            nc.scalar.activation(ea[:], lg[:], AF.Exp,
                                                 bias=bias, scale=scl)
                            eas.append(ea)
                        for np2 in range(2):
                            o_ps = [ps.tile([128, 512], F32, tag="b", name="ops")
                                    for _ in range(2)]
                            s_ps = [ps.tile([128, 512], F32, tag="b", name="sps")
                                    for _ in range(2)]
                            for mi in range(8):
                                for k in range(2):
                                    lhsT = eas[mi][:, 128 * (2 * np2 + k):
                                                   128 * (2 * np2 + k) + 128]
                                    nc.tensor.matmul(o_ps[k][:], lhsT,
                                                     vnat[:, mi, :],
                                                     start=(mi == 0),
                                                     stop=(mi == 7))
                                    nc.tensor.matmul(s_ps[k][:, 0:2], lhsT,
                                                     ones2[:],
                                                     start=(mi == 0),
                                                     stop=(mi == 7))
                            for k in range(2):
                                nck = 4 * nh + 2 * np2 + k
                                rec = sm.tile([128, 4], F32, name="rec")
                                nc.vector.reciprocal(rec[:, 0:1],
                                                     s_ps[k][:, 0:1])
                                if kind == "self":
                                    nc.scalar.mul(
                                        xnow_t[:, nck, :], o_ps[k][:],
                                        rec[:, 0:1])
                                elif kind == "avg":
                                    w = sm.tile([128, 4], F32, name="bw")
                                    nc.vector.tensor_tensor(
                                        w[:, 0:1], rec[:, 0:1],
                                        pars_bc[:, 0:1], op=ALU.mult)
                                    nc.scalar.mul(
                                        xprev_t[:, nck, :], o_ps[k][:],
                                        w[:, 0:1])
                                else:
                                    w = sm.tile([128, 4], F32, name="bw")
                                    nc.vector.tensor_tensor(
                                        w[:, 0:1], rec[:, 0:1],
                                        pars_bc[:, 1:2], op=ALU.mult)
                                    t_ = scr.tile([128, 512], F32, tag="s", name="mx")
                                    nc.vector.tensor_scalar_mul(
                                        t_[:], o_ps[k][:], w[:, 0:1])
                                    nc.vector.tensor_tensor(
                                        xprev_t[:, nck, :],
                                        xprev_t[:, nck, :], t_[:], op=ALU.add)

                # ---- per-image emission order: x block first (PE ramps
                # on transposes/qproj/self-attn), px chunks interleaved
                # between self-attention halves so their DMA/DVE hides
                # under PE work. Pool stage-2 runs on idle gpsimd.
                avgT_t = ap_.tile([128, 4, N], F32R, tag="avgT", name="avgT")
                maxT_t = ap_.tile([128, 4, N], F32R, tag="maxT", name="maxT")

                def px_chunk(ch):
                    pxc = ap_.tile([128, 2, 512], F32R, tag="pxc", bufs=2,
                                   name="pxc")
                    pls = []
                    for kk in range(4):
                        pl = ld.tile([128, PC], F32R, tag="pxload",
                                     bufs=6, name="pl")
                        nc.sync.dma_start(
                            pl[:],
                            px_d[img, 512 * ch + 128 * kk:
                                 512 * ch + 128 * kk + 128, :])
                        pls.append(pl)
                    for pc in range(2):
                        ptb = ps.tile([128, 512], F32R, tag="b", name="ptb")
                        for kk in range(4):
                            nc.tensor.transpose(
                                ptb[:, 128 * kk:128 * kk + 128],
                                pls[kk][:, 128 * pc:128 * pc + 128], ident[:])
                        nc.scalar.copy(pxc[:, pc, :], ptb[:])
                    for ci in range(4):
                        pq = ps.tile([128, 512], F32, tag="b", name="pq")
                        for pc in range(2):
                            nc.tensor.matmul(
                                pq[:], wp_s[:, pc, 128 * ci:128 * ci + 128],
                                pxc[:, pc, :],
                                start=(pc == 0), stop=(pc == 1))
                        # 2x2 pooling: one XY-reduce per path, straight
                        # from PSUM (avgT holds 4*mean; 0.25 folded into
                        # exp scale and host pars)
                        vb = pq[:].rearrange("p (i a j b) -> p i j a b",
                                             i=4, a=2, j=32, b=2)
                        nc.vector.tensor_reduce(
                            maxT_t[:, ci, 128 * ch:128 * ch + 128]
                            .rearrange("p (i j) -> p i j", j=32),
                            vb, axis=mybir.AxisListType.XY, op=ALU.max)
                        with nc.allow_low_precision(
                                reason="f32r is full fp32 bits"):
                            nc.vector.tensor_reduce(
                                avgT_t[:, ci, 128 * ch:128 * ch + 128]
                                .rearrange("p (i j) -> p i j", j=32),
                                vb, axis=mybir.AxisListType.XY, op=ALU.add)

                # ---- xT (batched PE transposes), spilled to DRAM; qT proj
                xT_t = ap_.tile([128, 4, N], F32R, tag="xTtr", name="xTtr")
                for nt in range(8):
                    xl = ld.tile([128, C], F32R, tag="xload", name="xl")
                    nc.sync.dma_start(xl[:], x_d[img, 128 * nt:128 * nt + 128, :])
                    xtb = ps.tile([128, 512], F32R, tag="b", name="xtb")
                    for ic in range(4):
                        nc.tensor.transpose(xtb[:, 128 * ic:128 * ic + 128],
                                            xl[:, 128 * ic:128 * ic + 128],
                                            ident[:])
                    nc.scalar.copy(
                        xT_t[:, :, 128 * nt:128 * nt + 128],
                        xtb[:].rearrange("p (ic q) -> p ic q", q=128))
                for ic in range(4):
                    nc.sync.dma_start(xT_d[img, ic], xT_t[:, ic, :])
                for ci in range(4):
                    for nh in range(2):
                        qp = ps.tile([128, 512], F32, tag="b", name="qp")
                        for ic in range(4):
                            nc.tensor.matmul(
                                qp[:], wq_s[:, ic, 128 * ci:128 * ci + 128],
                                xT_t[:, ic, 512 * nh:512 * nh + 512],
                                start=(ic == 0), stop=(ic == 3))
                        nc.scalar.copy(qT_t[:, ci, 512 * nh:512 * nh + 512],
                                       qp[:])
                # ---- qkv natural (tag "nat" slot shared with avg/max nat)
                nat_t = ap_.tile([128, 8, C], F32R, tag="nat", name="qkvnat")
                for mi in range(8):
                    ntb = ps.tile([128, 512], F32R, tag="b", name="ntb")
                    for ci in range(4):
                        nc.tensor.transpose(ntb[:, 128 * ci:128 * ci + 128],
                                            qT_t[:, ci, 128 * mi:128 * mi + 128],
                                            ident[:])
                    nc.scalar.copy(nat_t[:, mi, :], ntb[:])
                for ch in range(4):
                    px_chunk(ch)
                do_attn("self", qT_t, nat_t, nhs=(0,))
                for ch in range(4, 8):
                    px_chunk(ch)
                do_attn("self", qT_t, nat_t, nhs=(1,))

                # ---- avg attention
                nat_t = ap_.tile([128, 8, C], F32R, tag="nat", name="avgnat")
                for mi in range(8):
                    ntb = ps.tile([128, 512], F32R, tag="b", name="ntb")
                    for ci in range(4):
                        nc.tensor.transpose(ntb[:, 128 * ci:128 * ci + 128],
                                            avgT_t[:, ci, 128 * mi:128 * mi + 128],
                                            ident[:])
                    nc.scalar.copy(nat_t[:, mi, :], ntb[:])
                do_attn("avg", avgT_t, nat_t)
                # ---- max attention
                nat_t = ap_.tile([128, 8, C], F32R, tag="nat", name="maxnat")
                for mi in range(8):
                    ntb = ps.tile([128, 512], F32R, tag="b", name="ntb")
                    for ci in range(4):
                        nc.tensor.transpose(ntb[:, 128 * ci:128 * ci + 128],
                                            maxT_t[:, ci, 128 * mi:128 * mi + 128],
                                            ident[:])
                    nc.scalar.copy(nat_t[:, mi, :], ntb[:])
                do_attn("max", maxT_t, nat_t)

                # ---- fuse matmul; PSUM->SBUF copy carries BN1 partial sums
                for oi in range(4):
                    for v in range(2):
                        fp = ps.tile([128, 512], F32, tag="b", name="fp")
                        for ii in range(8):
                            rhs = (xnow_t[:, 4 * v + ii, :] if ii < 4
                                   else xprev_t[:, 4 * v + (ii - 4), :])
                            nc.tensor.matmul(
                                fp[:], fw_s[:, ii, 128 * oi:128 * oi + 128],
                                rhs, start=(ii == 0), stop=(ii == 7))
                        slot = 2 * img + v
                        nc.scalar.activation(
                            fusx_s[img][:, oi, v], fp[:], AF.Copy,
                            accum_out=s1acc[:, oi, slot:slot + 1])
                        fsq = scr.tile([128, 512], F32R, tag="s", name="fsq")
                        nc.vector.scalar_tensor_tensor(
                            fsq[:], fp[:], 1.0, fusx_s[img][:, oi, v],
                            op0=ALU.mult, op1=ALU.mult,
                            accum_out=ss1acc[:, oi, slot:slot + 1])

        # =================== conv scope ===================
        with tc.tile_pool(name="conv", bufs=1) as cp_:
            ow_s = cp_.tile([128, 9, 4, C], F32R, tag="ow", name="ow")
            nc.sync.dma_start(ow_s[:],
                              ow_d.rearrange("t (ic p) o -> p t ic o", p=128))
            # padded conv input, shared across images (pads zeroed once;
            # interior fully rewritten per image, WAR deps order it)
            x2T = cp_.tile([128, 4, 1156], F32R, tag="x2T", name="x2T")
            zrow = cp_.tile([128, 34], F32, tag="zrow", name="zrow")
            nc.gpsimd.memset(zrow[:], 0.0)
            x2g = x2T[:].rearrange("p ci (h w) -> p ci h w", w=34)
            for ci in range(4):
                nc.vector.tensor_copy(x2g[:, ci, 0, :], zrow[:])
                nc.vector.tensor_copy(x2g[:, ci, 33, :], zrow[:])
                nc.vector.tensor_copy(x2g[:, ci, :, 0], zrow[:])
                nc.vector.tensor_copy(x2g[:, ci, :, 33], zrow[:])

            # ---- BN1 global stats (AR overlaps ow load / memsets)
            sum1 = sm.tile([128, 4], F32, name="sum1")
            ssq1 = sm.tile([128, 4], F32, name="ssq1")
            nc.vector.tensor_reduce(sum1[:], s1acc[:], axis=X_AXIS, op=ALU.add)
            nc.vector.tensor_reduce(ssq1[:], ss1acc[:], axis=X_AXIS, op=ALU.add)
            nc.gpsimd.dma_start(bn1_in[:, 0:4], sum1[:])
            nc.gpsimd.dma_start(bn1_in[:, 4:8], ssq1[:])
            nc.gpsimd.collective_compute(
                "AllGather", ALU.bypass, replica_groups=[list(range(N_CORES))],
                ins=[bn1_in.opt()], outs=[bn1_all.opt()])
            allg = sm.tile([128, 8, 8], F32, name="allg")
            nc.sync.dma_start(allg[:],
                              bn1_all.rearrange("(c p) f -> p c f", p=128))
            allst = sm.tile([128, 8], F32, name="allst")
            nc.vector.tensor_reduce(allst[:],
                                    allg[:].rearrange("p c f -> p f c"),
                                    axis=X_AXIS, op=ALU.add)
            mean1 = sm.tile([128, 4], F32, name="mean1")
            tA = sm.tile([128, 4], F32, name="tA")
            tB = sm.tile([128, 4], F32, name="tB")
            nc.scalar.mul(mean1[:], allst[:, 0:4], INV_CNT)
            nc.scalar.mul(tA[:], allst[:, 4:8], INV_CNT)
            nc.scalar.square(tB[:], mean1[:])
            nc.vector.tensor_tensor(tA[:], tA[:], tB[:], op=ALU.subtract)
            nc.scalar.activation(tA[:], tA[:], AF.Sqrt, bias=eps_t[:])
            nc.vector.reciprocal(tA[:], tA[:])
            nc.vector.tensor_tensor(s1v[:], g1_s[:], tA[:], op=ALU.mult)
            nc.vector.tensor_tensor(tB[:], mean1[:], s1v[:], op=ALU.mult)
            nc.vector.tensor_tensor(t1v[:], b1_s[:], tB[:], op=ALU.subtract)

            # ---- per image: BN1 apply + residual, then conv 3x3
            # x2T[c, (h+1)*34 + (w+1)] = x[c, pos] + relu(s1*fusx + t1)
            # fusx[oi, v][c, u]: pos = 2u+v, u = 16h + w', w = 2w'+v
            ysb = [cp_.tile([128, 4, N], BF16, tag=f"ysb{img}",
                            name=f"ysb{img}") for img in range(IMGS)]
            for img in range(IMGS):
                xr = cp_.tile([128, 4, N], F32R, tag="xr", name="xr")
                nc.sync.dma_start(
                    xr[:], xT_d[img].rearrange("ci p n -> p ci n"))
                for oi in range(4):
                    x2v = x2T[:, oi].rearrange(
                        "p (h w2 two) -> p h w2 two", h=34, two=2)
                    xin = xr[:, oi].rearrange(
                        "p (h w2 two) -> p h w2 two", h=32, two=2)
                    for v in range(2):
                        rt = scr.tile([128, 512], F32, tag="s", name="rt")
                        nc.scalar.activation(rt[:], fusx_s[img][:, oi, v],
                                             AF.Relu,
                                             bias=t1v[:, oi:oi + 1],
                                             scale=s1v[:, oi:oi + 1])
                        rtv = rt[:].rearrange("p (h w) -> p h w", w=16)
                        # dst col index = 1 + v + 2*w'  ->  (t, k0)
                        t_par = (1 + v) % 2
                        k0 = (1 + v) // 2
                        dst = x2v[:, 1:33, k0:k0 + 16, t_par]
                        nc.vector.tensor_tensor(dst, rtv,
                                                xin[:, :, :, v], op=ALU.add)

                x2r = [x2T[:, ci].rearrange("p (h w) -> p h w", w=34)
                       for ci in range(4)]
                for oc in range(4):
                    for half in range(2):
                        yp = ps.tile([128, 512], F32, tag="b", name="yp")
                        k = 0
                        for tap in range(9):
                            dh, dw = tap // 3, tap % 3
                            for ci in range(4):
                                rhs = x2r[ci][:, dh + 16 * half:
                                              dh + 16 * half + 16,
                                              dw:dw + 32]
                                nc.tensor.matmul(
                                    yp[:], ow_s[:, tap, ci,
                                                128 * oc:128 * oc + 128],
                                    rhs, start=(k == 0), stop=(k == 35))
                                k += 1
                        slot = 2 * img + half
                        dst = ysb[img][:, oc, 512 * half:512 * half + 512]
                        nc.scalar.activation(
                            dst, yp[:], AF.Copy,
                            accum_out=s2acc[:, oc, slot:slot + 1])
                        ysq = scr.tile([128, 512], F32R, tag="s", name="ysq")
                        nc.vector.scalar_tensor_tensor(
                            ysq[:], yp[:], 1.0, dst,
                            op0=ALU.mult, op1=ALU.mult,
                            accum_out=ss2acc[:, oc, slot:slot + 1])

            # ---- BN2 global stats
            sum2 = sm.tile([128, 4], F32, name="sum2")
            ssq2 = sm.tile([128, 4], F32, name="ssq2")
            nc.vector.tensor_reduce(sum2[:], s2acc[:], axis=X_AXIS, op=ALU.add)
            nc.vector.tensor_reduce(ssq2[:], ss2acc[:], axis=X_AXIS,
                                    op=ALU.add)
            nc.gpsimd.dma_start(bn2_in[:, 0:4], sum2[:])
            nc.gpsimd.dma_start(bn2_in[:, 4:8], ssq2[:])
            nc.gpsimd.collective_compute(
                "AllGather", ALU.bypass, replica_groups=[list(range(N_CORES))],
                ins=[bn2_in.opt()], outs=[bn2_all.opt()])
            allg2 = sm.tile([128, 8, 8], F32, name="allg2")
            nc.sync.dma_start(allg2[:],
                              bn2_all.rearrange("(c p) f -> p c f", p=128))
            allst2 = sm.tile([128, 8], F32, name="allst2")
            nc.vector.tensor_reduce(allst2[:],
                                    allg2[:].rearrange("p c f -> p f c"),
                                    axis=X_AXIS, op=ALU.add)
            mean2 = sm.tile([128, 4], F32, name="mean2")
            uA = sm.tile([128, 4], F32, name="uA")
            uB = sm.tile([128, 4], F32, name="uB")
            nc.scalar.mul(mean2[:], allst2[:, 0:4], INV_CNT)
            nc.scalar.mul(uA[:], allst2[:, 4:8], INV_CNT)
            nc.scalar.square(uB[:], mean2[:])
            nc.vector.tensor_tensor(uA[:], uA[:], uB[:], op=ALU.subtract)
            nc.scalar.activation(uA[:], uA[:], AF.Sqrt, bias=eps_t[:])
            nc.vector.reciprocal(uA[:], uA[:])
            nc.vector.tensor_tensor(s2v[:], g2_s[:], uA[:], op=ALU.mult)
            nc.vector.tensor_tensor(uB[:], mean2[:], s2v[:], op=ALU.mult)
            nc.vector.tensor_tensor(t2v[:], b2_s[:], uB[:], op=ALU.subtract)

            # ---- BN2 apply (per-partition Act) + transpose to natural + store
            for img in range(IMGS):
                w1T = cp_.tile([128, 4, N], F32R, tag="w1T", name="w1T")
                for oc in range(4):
                    nc.scalar.activation(w1T[:, oc, :], ysb[img][:, oc, :],
                                         AF.Relu,
                                         bias=t2v[:, oc:oc + 1],
                                         scale=s2v[:, oc:oc + 1])
                for t in range(8):
                    pt = ps.tile([128, 512], F32R, tag="b", name="ot")
                    for oc in range(4):
                        nc.tensor.transpose(
                            pt[:, 128 * oc:128 * oc + 128],
                            w1T[:, oc, 128 * t:128 * t + 128], ident[:])
                    w1n = scr.tile([128, 512], F32, tag="s", name="w1n")
                    nc.vector.tensor_copy(w1n[:], pt[:])
                    nc.sync.dma_start(out_d[img, 128 * t:128 * t + 128, :],
                                      w1n[:])

    nc.compile()
    return nc


_STATE = {}


def _get_nc():
    if "nc" not in _STATE:
        _STATE["nc"] = build_nc()
    return _STATE["nc"]


def make_in_maps(x, prevx, w_prev_qkv, w_qkv, fuse_w, fuse_b, bn1_g, bn1_b,
                 out_w, out_b, bn2_g, bn2_b, gamma, beta):
    f = np.float32
    wq = np.ascontiguousarray(np.asarray(w_qkv, f).T)
    wp = np.ascontiguousarray(np.asarray(w_prev_qkv, f).T)
    fw = np.ascontiguousarray(np.asarray(fuse_w, f))
    ow = np.ascontiguousarray(np.asarray(out_w, f).reshape(9, C, C))
    g = float(np.asarray(gamma, f).reshape(-1)[0])
    g1 = np.ascontiguousarray((g * np.asarray(bn1_g, f)).reshape(4, 128).T)
    b1 = np.ascontiguousarray((g * np.asarray(bn1_b, f)).reshape(4, 128).T)
    g2 = np.ascontiguousarray(np.asarray(bn2_g, f).reshape(4, 128).T)
    b2 = np.ascontiguousarray(np.asarray(bn2_b, f).reshape(4, 128).T)
    bt = float(np.asarray(beta, f).reshape(-1)[0])
    pars = np.array([[0.25 * bt, 1.0 - bt]], f)
    xf = np.asarray(x, f).reshape(16, N, C)
    pxf = np.asarray(prevx, f).reshape(16, MP, PC)
    maps = []
    for c in range(N_CORES):
        maps.append({
            "x": np.ascontiguousarray(xf[2 * c:2 * c + 2]),
            "px": np.ascontiguousarray(pxf[2 * c:2 * c + 2]),
            "wq": wq, "wp": wp, "fw": fw, "ow": ow,
            "g1": g1, "b1": b1, "g2": g2, "b2": b2, "pars": pars,
        })
    return maps


def kernel(**inputs):
    nc = _get_nc()
    maps = make_in_maps(**inputs)
    res = run_bass_kernel_spmd(nc, maps, list(range(N_CORES)))
    out = np.concatenate([res.results[c]["out"] for c in range(N_CORES)],
                         axis=0)
    return out.reshape(16, 32, 32, C).astype(np.float32)


# revision 19
# speedup vs baseline: 1.0033x; 1.0033x over previous
"""Self-contained Trainium2 Bass kernel for nn_CrossStageAttention.

Data-parallel over batch: 16 images -> 8 NeuronCores x 2 images each.
Training-mode BatchNorm statistics are made global via two tiny AllReduces.

All heavy matmuls run as float32r on the PE array. The torch
"(attn@v).transpose(1,2).reshape" scramble is absorbed into the fuse access
patterns (o_nat orientation): catT[i, pos=2u+v] = o_nat[512v+i, u].

v2: the whole post-attention pipeline is channel-on-partition:
 - fusx / xT stay in SBUF (no DRAM spills)
 - conv input is one 2D-padded buffer x2T[c, (h+1)*34 + (w+1)]
 - conv output yT[c_out, pos] via lhsT=ow, rhs=x2T window; BN2 stats fold
   into the PSUM->SBUF copies (Act accum_out / DVE scalar_tensor_tensor)
 - BN2 apply is one per-partition Act relu(scale*y+bias) per channel chunk
 - final PE transposes produce the natural [pos, C] output
"""
import numpy as np
from contextlib import ExitStack

import concourse.bass as bass
import concourse.tile as tile
import concourse.bacc as bacc
from concourse import mybir, masks
from concourse.bass_utils import run_bass_kernel_spmd

N_CORES = 8
IMGS = 2
C = 512
N = 1024          # query positions per image (32x32)
PC = 256
MP = 4096         # prev positions per image (64x64)
F32 = mybir.dt.float32
F32R = mybir.dt.float32r
BF16 = mybir.dt.bfloat16
SCALE = 32 ** -0.5
B0_SELF = 128.0   # constant softmax-stabilization bias for self-attention
EPS = 1e-5
INV_CNT = 1.0 / (16 * 1024)
AF = mybir.ActivationFunctionType
ALU = mybir.AluOpType
X_AXIS = mybir.AxisListType.X


def build_nc():
    nc = bacc.Bacc("TRN2", target_bir_lowering=False, debug=False,
                   num_devices=N_CORES)
    x_d = nc.dram_tensor("x", [IMGS, N, C], F32R, kind="ExternalInput").ap()
    px_d = nc.dram_tensor("px", [IMGS, MP, PC], F32R, kind="ExternalInput").ap()
    wq_d = nc.dram_tensor("wq", [C, C], F32R, kind="ExternalInput").ap()
    wp_d = nc.dram_tensor("wp", [PC, C], F32R, kind="ExternalInput").ap()
    fw_d = nc.dram_tensor("fw", [2 * C, C], F32R, kind="ExternalInput").ap()
    ow_d = nc.dram_tensor("ow", [9, C, C], F32R, kind="ExternalInput").ap()
    g1_d = nc.dram_tensor("g1", [128, 4], F32, kind="ExternalInput").ap()
    b1_d = nc.dram_tensor("b1", [128, 4], F32, kind="ExternalInput").ap()
    g2_d = nc.dram_tensor("g2", [128, 4], F32, kind="ExternalInput").ap()
    b2_d = nc.dram_tensor("b2", [128, 4], F32, kind="ExternalInput").ap()
    pars_d = nc.dram_tensor("pars", [1, 2], F32, kind="ExternalInput").ap()
    out_d = nc.dram_tensor("out", [IMGS, N, C], F32, kind="ExternalOutput").ap()

    with tile.TileContext(nc) as tc, ExitStack() as ctx:
        const = ctx.enter_context(tc.tile_pool(name="const", bufs=1))
        per = ctx.enter_context(tc.tile_pool(name="per", bufs=1))
        scr = ctx.enter_context(tc.tile_pool(name="scr", bufs=10))   # [128,512] scratch
        ld = ctx.enter_context(tc.tile_pool(name="ld", bufs=3))
        sm = ctx.enter_context(tc.tile_pool(name="sm", bufs=10))
        ps = ctx.enter_context(tc.tile_pool(name="ps", bufs=8, space="PSUM"))
        dram = ctx.enter_context(tc.tile_pool(name="dram", bufs=1, space="DRAM"))

        # ------------- DRAM scratch (collective payloads only) -------------
        bn1_in = dram.tile([128, 8], F32, tag="bn1i")
        bn1_all = dram.tile([128 * N_CORES, 8], F32, tag="bn1o")
        bn2_in = dram.tile([128, 8], F32, tag="bn2i")
        bn2_all = dram.tile([128 * N_CORES, 8], F32, tag="bn2o")

        # ------------- persistent cross-scope tensors -------------
        xT_d = dram.tile([IMGS, 4, 128, N], F32R, tag="xT_d")
        fusx_s = [per.tile([128, 4, 2, 512], BF16, tag=f"fusx{i}",
                           name=f"fusx{i}") for i in range(IMGS)]  # 2 MB

        # ------------- constants / params -------------
        identF = const.tile([128, 128], F32, tag="identF")
        masks.make_identity(nc, identF[:])
        ident = const.tile([128, 128], F32R, tag="ident")
        nc.vector.tensor_copy(ident[:], identF[:])
        onesF = const.tile([128, 2], F32, tag="onesF")
        nc.gpsimd.memset(onesF[:], 1.0)
        ones2 = const.tile([128, 2], F32R, tag="ones2")
        nc.vector.tensor_copy(ones2[:], onesF[:])
        b0s = const.tile([128, 1], F32, tag="b0s")
        nc.gpsimd.memset(b0s[:], -B0_SELF)
        eps_t = const.tile([128, 1], F32, tag="eps")
        nc.gpsimd.memset(eps_t[:], EPS)
        g1_s = const.tile([128, 4], F32, tag="g1")
        b1_s = const.tile([128, 4], F32, tag="b1")
        g2_s = const.tile([128, 4], F32, tag="g2")
        b2_s = const.tile([128, 4], F32, tag="b2")
        pars_s = const.tile([1, 2], F32, tag="pars")
        pars_bc = const.tile([128, 2], F32, tag="parsbc")
        s1acc = const.tile([128, 4, 4], F32, tag="s1acc")
        ss1acc = const.tile([128, 4, 4], F32, tag="ss1acc")
        s2acc = const.tile([128, 4, 4], F32, tag="s2acc")
        ss2acc = const.tile([128, 4, 4], F32, tag="ss2acc")
        s1v = const.tile([128, 4], F32, tag="s1v")
        t1v = const.tile([128, 4], F32, tag="t1v")
        s2v = const.tile([128, 4], F32, tag="s2v")
        t2v = const.tile([128, 4], F32, tag="t2v")
        nc.sync.dma_start(g1_s[:], g1_d)
        nc.sync.dma_start(b1_s[:], b1_d)
        nc.sync.dma_start(g2_s[:], g2_d)
        nc.sync.dma_start(b2_s[:], b2_d)
        nc.sync.dma_start(pars_s[:], pars_d)
        nc.gpsimd.partition_broadcast(pars_bc[:], pars_s[:])

        def transpose_to(dst_ap, src_ap, eng):
            pt = ps.tile([128, 512], F32R, tag="b", name="tp")
            nc.tensor.transpose(pt[:, 0:128], src_ap, ident[:])
            if eng == "act":
                nc.scalar.copy(dst_ap, pt[:, 0:128])
            else:
                nc.vector.tensor_copy(dst_ap, pt[:, 0:128])

        # =================== attention scope ===================
        with tc.tile_pool(name="attn", bufs=1) as ap_:
            wq_s = ap_.tile([128, 4, C], F32R, tag="wq", name="wq")
            wp_s = ap_.tile([128, 2, C], F32R, tag="wp", name="wp")
            fw_s = ap_.tile([128, 8, C], F32R, tag="fw", name="fw")
            nc.sync.dma_start(wq_s[:], wq_d.rearrange("(ic p) c -> p ic c", p=128))
            nc.sync.dma_start(wp_s[:], wp_d.rearrange("(ic p) c -> p ic c", p=128))
            qT_t = None
            for img in range(IMGS):
                qT_t = ap_.tile([128, 4, N], F32R, tag="qT", name="qT")
                xnow_t = ap_.tile([128, 8, C], F32R, tag="xnow", name="xnow")
                xprev_t = ap_.tile([128, 8, C], F32R, tag="xprev", name="xprev")

                def do_attn(kind, kvT, vnat, nhs=(0, 1)):
                    bias = b0s[:] if kind == "self" else 0.0
                    scl = SCALE * 0.25 if kind == "avg" else SCALE
                    for nh in nhs:
                        eas = []
                        for mi in range(8):
                            lg = ps.tile([128, 512], F32, tag="b", name="lg")
                            for ci in range(4):
                                nc.tensor.matmul(
                                    lg[:],
                                    kvT[:, ci, 128 * mi:128 * mi + 128],
                                    qT_t[:, ci, 512 * nh:512 * nh + 512],
                                    start=(ci == 0), stop=(ci == 3))
                            ea = scr.tile([128, 512], F32R, tag="s", name="ea")
                            nc.scalar.activation(ea[:], lg[:], AF.Exp,
                                                 bias=bias, scale=scl)
                            eas.append(ea)
                        for np2 in range(2):
                            o_ps = [ps.tile([128, 512], F32, tag="b", name="ops")
                                    for _ in range(2)]
                            s_ps = [ps.tile([128, 512], F32, tag="b", name="sps")
                                    for _ in range(2)]
                            for mi in range(8):
                                for k in range(2):
                                    lhsT = eas[mi][:, 128 * (2 * np2 + k):
                                                   128 * (2 * np2 + k) + 128]
                                    nc.tensor.matmul(o_ps[k][:], lhsT,
                                                     vnat[:, mi, :],
                                                     start=(mi == 0),
                                                     stop=(mi == 7))
                                    nc.tensor.matmul(s_ps[k][:, 0:2], lhsT,
                                                     ones2[:],
                                                     start=(mi == 0),
                                                     stop=(mi == 7))
                            for k in range(2):
                                nck = 4 * nh + 2 * np2 + k
                                rec = sm.tile([128, 4], F32, name="rec")
                                nc.vector.reciprocal(rec[:, 0:1],
                                                     s_ps[k][:, 0:1])
                                if kind == "self":
                                    nc.scalar.mul(
                                        xnow_t[:, nck, :], o_ps[k][:],
                                        rec[:, 0:1])
                                elif kind == "avg":
                                    w = sm.tile([128, 4], F32, name="bw")
                                    nc.vector.tensor_tensor(
                                        w[:, 0:1], rec[:, 0:1],
                                        pars_bc[:, 0:1], op=ALU.mult)
                                    nc.scalar.mul(
                                        xprev_t[:, nck, :], o_ps[k][:],
                                        w[:, 0:1])
                                else:
                                    w = sm.tile([128, 4], F32, name="bw")
                                    nc.vector.tensor_tensor(
                                        w[:, 0:1], rec[:, 0:1],
                                        pars_bc[:, 1:2], op=ALU.mult)
                                    t_ = scr.tile([128, 512], F32, tag="s", name="mx")
                                    nc.scalar.mul(
                                        t_[:], o_ps[k][:], w[:, 0:1])
                                    nc.vector.tensor_tensor(
                                        xprev_t[:, nck, :],
                                        xprev_t[:, nck, :], t_[:], op=ALU.add)

                # ---- per-image emission order: x block first (PE ramps
                # on transposes/qproj/self-attn), px chunks interleaved
                # between self-attention halves so their DMA/DVE hides
                # under PE work. Pool stage-2 runs on idle gpsimd.
                avgT_t = ap_.tile([128, 4, N], F32R, tag="avgT", name="avgT")
                maxT_t = ap_.tile([128, 4, N], F32R, tag="maxT", name="maxT")

                def px_chunk(ch):
                    pxc = ap_.tile([128, 2, 512], F32R, tag="pxc", bufs=2,
                                   name="pxc")
                    pls = []
                    for kk in range(4):
                        pl = ld.tile([128, PC], F32R, tag="pxload",
                                     bufs=6, name="pl")
                        nc.sync.dma_start(
                            pl[:],
                            px_d[img, 512 * ch + 128 * kk:
                                 512 * ch + 128 * kk + 128, :])
                        pls.append(pl)
                    for pc in range(2):
                        ptb = ps.tile([128, 512], F32R, tag="b", name="ptb")
                        for kk in range(4):
                            nc.tensor.transpose(
                                ptb[:, 128 * kk:128 * kk + 128],
                                pls[kk][:, 128 * pc:128 * pc + 128], ident[:])
                        nc.scalar.copy(pxc[:, pc, :], ptb[:])
                    for ci in range(4):
                        pq = ps.tile([128, 512], F32, tag="b", name="pq")
                        for pc in range(2):
                            nc.tensor.matmul(
                                pq[:], wp_s[:, pc, 128 * ci:128 * ci + 128],
                                pxc[:, pc, :],
                                start=(pc == 0), stop=(pc == 1))
                        # 2x2 pooling: one XY-reduce per path, straight
                        # from PSUM (avgT holds 4*mean; 0.25 folded into
                        # exp scale and host pars)
                        vb = pq[:].rearrange("p (i a j b) -> p i j a b",
                                             i=4, a=2, j=32, b=2)
                        nc.vector.tensor_reduce(
                            maxT_t[:, ci, 128 * ch:128 * ch + 128]
                            .rearrange("p (i j) -> p i j", j=32),
                            vb, axis=mybir.AxisListType.XY, op=ALU.max)
                        with nc.allow_low_precision(
                                reason="f32r is full fp32 bits"):
                            nc.vector.tensor_reduce(
                                avgT_t[:, ci, 128 * ch:128 * ch + 128]
                                .rearrange("p (i j) -> p i j", j=32),
                                vb, axis=mybir.AxisListType.XY, op=ALU.add)

                # ---- xT (batched PE transposes), spilled to DRAM; qT proj
                xT_t = ap_.tile([128, 4, N], F32R, tag="xTtr", name="xTtr")
                for nt in range(8):
                    xl = ld.tile([128, C], F32R, tag="xload", name="xl")
                    nc.sync.dma_start(xl[:], x_d[img, 128 * nt:128 * nt + 128, :])
                    xtb = ps.tile([128, 512], F32R, tag="b", name="xtb")
                    for ic in range(4):
                        nc.tensor.transpose(xtb[:, 128 * ic:128 * ic + 128],
                                            xl[:, 128 * ic:128 * ic + 128],
                                            ident[:])
                    nc.scalar.copy(
                        xT_t[:, :, 128 * nt:128 * nt + 128],
                        xtb[:].rearrange("p (ic q) -> p ic q", q=128))
                for ic in range(4):
                    nc.sync.dma_start(xT_d[img, ic], xT_t[:, ic, :])
                for ci in range(4):
                    for nh in range(2):
                        qp = ps.tile([128, 512], F32, tag="b", name="qp")
                        for ic in range(4):
                            nc.tensor.matmul(
                                qp[:], wq_s[:, ic, 128 * ci:128 * ci + 128],
                                xT_t[:, ic, 512 * nh:512 * nh + 512],
                                start=(ic == 0), stop=(ic == 3))
                        nc.scalar.copy(qT_t[:, ci, 512 * nh:512 * nh + 512],
                                       qp[:])
                # ---- qkv natural (tag "nat" slot shared with avg/max nat)
                nat_t = ap_.tile([128, 8, C], F32R, tag="nat", name="qkvnat")
                for mi in range(8):
                    ntb = ps.tile([128, 512], F32R, tag="b", name="ntb")
                    for ci in range(4):
                        nc.tensor.transpose(ntb[:, 128 * ci:128 * ci + 128],
                                            qT_t[:, ci, 128 * mi:128 * mi + 128],
                                            ident[:])
                    nc.scalar.copy(nat_t[:, mi, :], ntb[:])
                if img == 0:
                    nc.sync.dma_start(
                        fw_s[:], fw_d.rearrange("(ic p) o -> p ic o", p=128))
                for ch in range(4):
                    px_chunk(ch)
                do_attn("self", qT_t, nat_t, nhs=(0,))
                for ch in range(4, 8):
                    px_chunk(ch)
                do_attn("self", qT_t, nat_t, nhs=(1,))

                # ---- avg attention
                nat_t = ap_.tile([128, 8, C], F32R, tag="nat", name="avgnat")
                for mi in range(8):
                    ntb = ps.tile([128, 512], F32R, tag="b", name="ntb")
                    for ci in range(4):
                        nc.tensor.transpose(ntb[:, 128 * ci:128 * ci + 128],
                                            avgT_t[:, ci, 128 * mi:128 * mi + 128],
                                            ident[:])
                    nc.scalar.copy(nat_t[:, mi, :], ntb[:])
                do_attn("avg", avgT_t, nat_t)
                # ---- max attention
                nat_t = ap_.tile([128, 8, C], F32R, tag="nat", name="maxnat")
                for mi in range(8):
                    ntb = ps.tile([128, 512], F32R, tag="b", name="ntb")
                    for ci in range(4):
                        nc.tensor.transpose(ntb[:, 128 * ci:128 * ci + 128],
                                            maxT_t[:, ci, 128 * mi:128 * mi + 128],
                                            ident[:])
                    nc.scalar.copy(nat_t[:, mi, :], ntb[:])
                do_attn("max", maxT_t, nat_t)

                # ---- fuse matmul; PSUM->SBUF copy carries BN1 partial sums
                for oi in range(4):
                    for v in range(2):
                        fp = ps.tile([128, 512], F32, tag="b", name="fp")
                        for ii in range(8):
                            rhs = (xnow_t[:, 4 * v + ii, :] if ii < 4
                                   else xprev_t[:, 4 * v + (ii - 4), :])
                            nc.tensor.matmul(
                                fp[:], fw_s[:, ii, 128 * oi:128 * oi + 128],
                                rhs, start=(ii == 0), stop=(ii == 7))
                        slot = 2 * img + v
                        nc.scalar.activation(
                            fusx_s[img][:, oi, v], fp[:], AF.Copy,
                            accum_out=s1acc[:, oi, slot:slot + 1])
                        fsq = scr.tile([128, 512], F32R, tag="s", name="fsq")
                        nc.vector.scalar_tensor_tensor(
                            fsq[:], fp[:], 1.0, fusx_s[img][:, oi, v],
                            op0=ALU.mult, op1=ALU.mult,
                            accum_out=ss1acc[:, oi, slot:slot + 1])

        # =================== conv scope ===================
        with tc.tile_pool(name="conv", bufs=1) as cp_:
            ow_s = cp_.tile([128, 9, 4, C], F32R, tag="ow", name="ow")
            nc.sync.dma_start(ow_s[:],
                              ow_d.rearrange("t (ic p) o -> p t ic o", p=128))
            # padded conv input, shared across images (pads zeroed once;
            # interior fully rewritten per image, WAR deps order it)
            x2T = cp_.tile([128, 4, 1156], F32R, tag="x2T", name="x2T")
            zrow = cp_.tile([128, 34], F32, tag="zrow", name="zrow")
            nc.gpsimd.memset(zrow[:], 0.0)
            x2g = x2T[:].rearrange("p ci (h w) -> p ci h w", w=34)
            for ci in range(4):
                nc.vector.tensor_copy(x2g[:, ci, 0, :], zrow[:])
                nc.vector.tensor_copy(x2g[:, ci, 33, :], zrow[:])
                nc.vector.tensor_copy(x2g[:, ci, :, 0], zrow[:])
                nc.vector.tensor_copy(x2g[:, ci, :, 33], zrow[:])

            # ---- BN1 global stats (AR overlaps ow load / memsets)
            sum1 = sm.tile([128, 4], F32, name="sum1")
            ssq1 = sm.tile([128, 4], F32, name="ssq1")
            nc.vector.tensor_reduce(sum1[:], s1acc[:], axis=X_AXIS, op=ALU.add)
            nc.vector.tensor_reduce(ssq1[:], ss1acc[:], axis=X_AXIS, op=ALU.add)
            nc.gpsimd.dma_start(bn1_in[:, 0:4], sum1[:])
            nc.gpsimd.dma_start(bn1_in[:, 4:8], ssq1[:])
            nc.gpsimd.collective_compute(
                "AllGather", ALU.bypass, replica_groups=[list(range(N_CORES))],
                ins=[bn1_in.opt()], outs=[bn1_all.opt()])
            allg = sm.tile([128, 8, 8], F32, name="allg")
            nc.sync.dma_start(allg[:],
                              bn1_all.rearrange("(c p) f -> p c f", p=128))
            allst = sm.tile([128, 8], F32, name="allst")
            nc.vector.tensor_reduce(allst[:],
                                    allg[:].rearrange("p c f -> p f c"),
                                    axis=X_AXIS, op=ALU.add)
            mean1 = sm.tile([128, 4], F32, name="mean1")
            tA = sm.tile([128, 4], F32, name="tA")
            tB = sm.tile([128, 4], F32, name="tB")
            nc.scalar.mul(mean1[:], allst[:, 0:4], INV_CNT)
            nc.scalar.mul(tA[:], allst[:, 4:8], INV_CNT)
            nc.scalar.square(tB[:], mean1[:])
            nc.vector.tensor_tensor(tA[:], tA[:], tB[:], op=ALU.subtract)
            nc.scalar.activation(tA[:], tA[:], AF.Sqrt, bias=eps_t[:])
            nc.vector.reciprocal(tA[:], tA[:])
            nc.vector.tensor_tensor(s1v[:], g1_s[:], tA[:], op=ALU.mult)
            nc.vector.tensor_tensor(tB[:], mean1[:], s1v[:], op=ALU.mult)
            nc.vector.tensor_tensor(t1v[:], b1_s[:], tB[:], op=ALU.subtract)

            # ---- per image: BN1 apply + residual, then conv 3x3
            # x2T[c, (h+1)*34 + (w+1)] = x[c, pos] + relu(s1*fusx + t1)
            # fusx[oi, v][c, u]: pos = 2u+v, u = 16h + w', w = 2w'+v
            ysb = [cp_.tile([128, 4, N], BF16, tag=f"ysb{img}",
                            name=f"ysb{img}") for img in range(IMGS)]
            for img in range(IMGS):
                xr = cp_.tile([128, 4, N], F32R, tag="xr", name="xr")
                nc.sync.dma_start(
                    xr[:], xT_d[img].rearrange("ci p n -> p ci n"))
                for oi in range(4):
                    x2v = x2T[:, oi].rearrange(
                        "p (h w2 two) -> p h w2 two", h=34, two=2)
                    xin = xr[:, oi].rearrange(
                        "p (h w2 two) -> p h w2 two", h=32, two=2)
                    for v in range(2):
                        rt = scr.tile([128, 512], F32, tag="s", name="rt")
                        nc.scalar.activation(rt[:], fusx_s[img][:, oi, v],
                                             AF.Relu,
                                             bias=t1v[:, oi:oi + 1],
                                             scale=s1v[:, oi:oi + 1])
                        rtv = rt[:].rearrange("p (h w) -> p h w", w=16)
                        # dst col index = 1 + v + 2*w'  ->  (t, k0)
                        t_par = (1 + v) % 2
                        k0 = (1 + v) // 2
                        dst = x2v[:, 1:33, k0:k0 + 16, t_par]
                        nc.vector.tensor_tensor(dst, rtv,
                                                xin[:, :, :, v], op=ALU.add)

                x2r = [x2T[:, ci].rearrange("p (h w) -> p h w", w=34)
                       for ci in range(4)]
                for oc in range(4):
                    for half in range(2):
                        yp = ps.tile([128, 512], F32, tag="b", name="yp")
                        k = 0
                        for tap in range(9):
                            dh, dw = tap // 3, tap % 3
                            for ci in range(4):
                                rhs = x2r[ci][:, dh + 16 * half:
                                              dh + 16 * half + 16,
                                              dw:dw + 32]
                                nc.tensor.matmul(
                                    yp[:], ow_s[:, tap, ci,
                                                128 * oc:128 * oc + 128],
                                    rhs, start=(k == 0), stop=(k == 35))
                                k += 1
                        slot = 2 * img + half
                        dst = ysb[img][:, oc, 512 * half:512 * half + 512]
                        nc.scalar.activation(
                            dst, yp[:], AF.Copy,
                            accum_out=s2acc[:, oc, slot:slot + 1])
                        ysq = scr.tile([128, 512], F32R, tag="s", name="ysq")
                        nc.vector.scalar_tensor_tensor(
                            ysq[:], yp[:], 1.0, dst,
                            op0=ALU.mult, op1=ALU.mult,
                            accum_out=ss2acc[:, oc, slot:slot + 1])

            # ---- BN2 global stats
            sum2 = sm.tile([128, 4], F32, name="sum2")
            ssq2 = sm.tile([128, 4], F32, name="ssq2")
            nc.vector.tensor_reduce(sum2[:], s2acc[:], axis=X_AXIS, op=ALU.add)
            nc.vector.tensor_reduce(ssq2[:], ss2acc[:], axis=X_AXIS,
                                    op=ALU.add)
            nc.gpsimd.dma_start(bn2_in[:, 0:4], sum2[:])
            nc.gpsimd.dma_start(bn2_in[:, 4:8], ssq2[:])
            nc.gpsimd.collective_compute(
                "AllGather", ALU.bypass, replica_groups=[list(range(N_CORES))],
                ins=[bn2_in.opt()], outs=[bn2_all.opt()])
            allg2 = sm.tile([128, 8, 8], F32, name="allg2")
            nc.sync.dma_start(allg2[:],
                              bn2_all.rearrange("(c p) f -> p c f", p=128))
            allst2 = sm.tile([128, 8], F32, name="allst2")
            nc.vector.tensor_reduce(allst2[:],
                                    allg2[:].rearrange("p c f -> p f c"),
                                    axis=X_AXIS, op=ALU.add)
            mean2 = sm.tile([128, 4], F32, name="mean2")
            uA = sm.tile([128, 4], F32, name="uA")
            uB = sm.tile([128, 4], F32, name="uB")
            nc.scalar.mul(mean2[:], allst2[:, 0:4], INV_CNT)
            nc.scalar.mul(uA[:], allst2[:, 4:8], INV_CNT)
            nc.scalar.square(uB[:], mean2[:])
            nc.vector.tensor_tensor(uA[:], uA[:], uB[:], op=ALU.subtract)
            nc.scalar.activation(uA[:], uA[:], AF.Sqrt, bias=eps_t[:])
            nc.vector.reciprocal(uA[:], uA[:])
            nc.vector.tensor_tensor(s2v[:], g2_s[:], uA[:], op=ALU.mult)
            nc.vector.tensor_tensor(uB[:], mean2[:], s2v[:], op=ALU.mult)
            nc.vector.tensor_tensor(t2v[:], b2_s[:], uB[:], op=ALU.subtract)

            # ---- BN2 apply (per-partition Act) + transpose to natural + store
            for img in range(IMGS):
                w1T = cp_.tile([128, 4, N], F32R, tag="w1T", name="w1T")
                for oc in range(4):
                    nc.scalar.activation(w1T[:, oc, :], ysb[img][:, oc, :],
                                         AF.Relu,
                                         bias=t2v[:, oc:oc + 1],
                                         scale=s2v[:, oc:oc + 1])
                for t in range(8):
                    pt = ps.tile([128, 512], F32R, tag="b", name="ot")
                    for oc in range(4):
                        nc.tensor.transpose(
                            pt[:, 128 * oc:128 * oc + 128],
                            w1T[:, oc, 128 * t:128 * t + 128], ident[:])
                    w1n = scr.tile([128, 512], F32, tag="s", name="w1n")
                    nc.vector.tensor_copy(w1n[:], pt[:])
                    nc.sync.dma_start(out_d[img, 128 * t:128 * t + 128, :],
                                      w1n[:])

    nc.compile()
    return nc


_STATE = {}


def _get_nc():
    if "nc" not in _STATE:
        _STATE["nc"] = build_nc()
    return _STATE["nc"]


def make_in_maps(x, prevx, w_prev_qkv, w_qkv, fuse_w, fuse_b, bn1_g, bn1_b,
                 out_w, out_b, bn2_g, bn2_b, gamma, beta):
    f = np.float32
    wq = np.ascontiguousarray(np.asarray(w_qkv, f).T)
    wp = np.ascontiguousarray(np.asarray(w_prev_qkv, f).T)
    fw = np.ascontiguousarray(np.asarray(fuse_w, f))
    ow = np.ascontiguousarray(np.asarray(out_w, f).reshape(9, C, C))
    g = float(np.asarray(gamma, f).reshape(-1)[0])
    g1 = np.ascontiguousarray((g * np.asarray(bn1_g, f)).reshape(4, 128).T)
    b1 = np.ascontiguousarray((g * np.asarray(bn1_b, f)).reshape(4, 128).T)
    g2 = np.ascontiguousarray(np.asarray(bn2_g, f).reshape(4, 128).T)
    b2 = np.ascontiguousarray(np.asarray(bn2_b, f).reshape(4, 128).T)
    bt = float(np.asarray(beta, f).reshape(-1)[0])
    pars = np.array([[0.25 * bt, 1.0 - bt]], f)
    xf = np.asarray(x, f).reshape(16, N, C)
    pxf = np.asarray(prevx, f).reshape(16, MP, PC)
    maps = []
    for c in range(N_CORES):
        maps.append({
            "x": np.ascontiguousarray(xf[2 * c:2 * c + 2]),
            "px": np.ascontiguousarray(pxf[2 * c:2 * c + 2]),
            "wq": wq, "wp": wp, "fw": fw, "ow": ow,
            "g1": g1, "b1": b1, "g2": g2, "b2": b2, "pars": pars,
        })
    return maps


def kernel(**inputs):
    nc = _get_nc()
    maps = make_in_maps(**inputs)
    res = run_bass_kernel_spmd(nc, maps, list(range(N_CORES)))
    out = np.concatenate([res.results[c]["out"] for c in range(N_CORES)],
                         axis=0)
    return out.reshape(16, 32, 32, C).astype(np.float32)
